# revision 5
# baseline (speedup 1.0000x reference)
"""MoE (top-2 of 8 experts, B=8192, D=2048) on 8 Trainium2 NeuronCores.

Strategy (expert-parallel, per sharding hint): the host computes the gate
softmax + top-2 routing (float64 numpy; rank-2/3 margins are ~3e-5 so the
selection matches any f32 reference platform), dispatches each token's rows
to its experts' cores, and each core computes
    y_e = relu(x_e @ W[e].T + b[e]) * gate_scale
for its gathered tokens as an fp16 tiled matmul on the PE array.  The host
then scatter-adds the (at most 2) expert contributions per token.

Device work per core: ~C*D*D*2 ~ 18 GFLOP fp16 (vs 8x that for the dense
reference), PSUM-accumulated over the 2048-deep contraction in 16 chunks of
128, with bias-add (VectorE) + relu*scale (ScalarE, per-partition scale)
epilogue fused off the critical PE path.
"""

import math

import numpy as np
import ml_dtypes

B, D, E, TOP_K = 8192, 2048, 8, 2
N_CORES = 8
P = 128
KD = D // P  # 16 contraction chunks
NT = 4
NSZ = D // NT  # 512 output columns per psum tile

_F16 = np.float16

_nc_cache = {}


def _routing(x, Wg, bg):
    """Gate softmax + top-2 in float64; returns (idx [B,2] int, vals [B,2] f32)."""
    logits = x.astype(np.float64) @ Wg.astype(np.float64).T + bg.astype(np.float64)
    logits -= logits.max(-1, keepdims=True)
    eL = np.exp(logits)
    gate = eL / eL.sum(-1, keepdims=True)
    order = np.argsort(-gate, axis=-1, kind="stable")
    idx = order[:, :TOP_K]
    vals = np.take_along_axis(gate, idx, -1).astype(np.float32)
    return idx, vals


def _build(m_tiles, reps=1):
    """Build + compile the per-core Bass kernel for C = m_tiles*128 tokens."""
    import concourse.mybir as mybir
    import concourse.tile as tile
    from concourse import bacc

    nc = bacc.Bacc("TRN2", target_bir_lowering=False)
    C = m_tiles * P
    xt = nc.dram_tensor("xt", [P, m_tiles, KD, P], mybir.dt.float16, kind="ExternalInput")
    wt = nc.dram_tensor("wt", [P, NT, KD, NSZ], mybir.dt.float16, kind="ExternalInput")
    bias = nc.dram_tensor("bias", [P, D], mybir.dt.float32, kind="ExternalInput")
    scale = nc.dram_tensor("scale", [P, m_tiles], mybir.dt.float32, kind="ExternalInput")
    y = nc.dram_tensor("y", [C, D], mybir.dt.float32, kind="ExternalOutput")

    with tile.TileContext(nc) as tc:
        with (
            tc.tile_pool(name="wp", bufs=1) as wp,
            tc.tile_pool(name="xp", bufs=1) as xp,
            tc.tile_pool(name="cp", bufs=1) as cp,
            tc.tile_pool(name="op", bufs=6) as op_,
            tc.tile_pool(name="pp", bufs=8, space="PSUM") as pp,
        ):
            def epilogue(ps, bias_sb, scale_sb, m, n):
                ot = op_.tile([P, NSZ], mybir.dt.float32, tag="ot", name="ot")
                nc.vector.tensor_tensor(
                    ot[:], ps[:], bias_sb[:, n * NSZ:(n + 1) * NSZ], mybir.AluOpType.add
                )
                nc.scalar.activation(
                    ot[:], ot[:], mybir.ActivationFunctionType.Relu,
                    scale=scale_sb[:, m:m + 1],
                )
                nc.sync.dma_start(y[m * P:(m + 1) * P, n * NSZ:(n + 1) * NSZ], ot[:])

            def body(_i=None):
                bias_sb = cp.tile([P, D], mybir.dt.float32, tag="bias", name="bias_sb")
                nc.sync.dma_start(bias_sb[:], bias[:])
                scale_sb = cp.tile([P, m_tiles], mybir.dt.float32, tag="scale", name="scale_sb")
                nc.sync.dma_start(scale_sb[:], scale[:])

                wts = [None] * NT
                wts[0] = wp.tile([P, KD, NSZ], mybir.dt.float16, tag="wt0", name="wt_sb0")
                nc.sync.dma_start(wts[0][:], wt[:, 0])

                # PE warmup while the first weight DMA is in flight: ~4us of
                # garbage matmuls un-throttle the HAM clock gate (1.2->2.4GHz)
                # so the real sweep starts warm.
                warm = cp.tile([P, 640], mybir.dt.float16, tag="warm", name="warm")
                nc.vector.memset(warm[:], 0.0)
                wps = pp.tile([P, NSZ], mybir.dt.float32, tag="ps", name="warmps")
                for _w in range(16):
                    nc.tensor.matmul(wps[:], warm[:, 0:P], warm[:, P:P + NSZ],
                                     start=True, stop=True)

                # Phase 1: n=0 sweep over all m-tiles.  PE starts after only
                # wt[0] (2MB) + xt[0] (0.5MB) land; the remaining weight and
                # token DMAs hide under this ~60us sweep.  xt loads go via the
                # ACT HWDGE ring so they don't FIFO behind wt on the SP ring.
                xts = [None] * m_tiles
                for m in range(m_tiles):
                    xts[m] = xp.tile([P, KD, P], mybir.dt.float16, tag=f"xt{m}", name=f"xt_sb{m}")
                    nc.scalar.dma_start(xts[m][:], xt[:, m])
                    ps = pp.tile([P, NSZ], mybir.dt.float32, tag="ps", name="ps")
                    for kd in range(KD):
                        nc.tensor.matmul(
                            ps[:], xts[m][:, kd], wts[0][:, kd],
                            start=(kd == 0), stop=(kd == KD - 1),
                        )
                    epilogue(ps, bias_sb, scale_sb, m, 0)
                for n in range(1, NT):
                    t = wp.tile([P, KD, NSZ], mybir.dt.float16, tag=f"wt{n}", name=f"wt_sb{n}")
                    nc.sync.dma_start(t[:], wt[:, n])
                    wts[n] = t
                # Phase 2: n=1..3 with 3 psum banks per m-tile.
                for m in range(m_tiles):
                    pss = [pp.tile([P, NSZ], mybir.dt.float32, tag="ps", name="ps") for _ in range(NT - 1)]
                    for kd in range(KD):
                        for j, n in enumerate(range(1, NT)):
                            nc.tensor.matmul(
                                pss[j][:], xts[m][:, kd], wts[n][:, kd],
                                start=(kd == 0), stop=(kd == KD - 1),
                            )
                    for j, n in enumerate(range(1, NT)):
                        epilogue(pss[j], bias_sb, scale_sb, m, n)

            if reps == 1:
                body()
            else:
                with tc.For_i(0, reps) as _i:
                    body(_i)

    nc.compile()
    return nc


def _get_nc(m_tiles, reps=1):
    key = (m_tiles, reps)
    if key not in _nc_cache:
        _nc_cache[key] = _build(m_tiles, reps)
    return _nc_cache[key]


def _prep_inputs(x, W, b, idx, vals):
    """Per-core input maps: blocked bf16 xT/wT layouts + bias/scale tiles."""
    in_maps = []
    token_lists = []
    counts = []
    for e in range(E):
        tok = np.where((idx == e).any(axis=1))[0]
        token_lists.append(tok)
        counts.append(len(tok))
    c_max = max(counts)
    m_tiles = max(1, math.ceil(c_max / P))
    C = m_tiles * P

    for e in range(E):
        tok = token_lists[e]
        cnt = len(tok)
        Xp = np.zeros((C, D), dtype=_F16)
        Xp[:cnt] = x[tok].astype(_F16)
        xt_np = np.ascontiguousarray(
            Xp.reshape(m_tiles, P, KD, P).transpose(3, 0, 2, 1)
        )
        wt_np = np.ascontiguousarray(
            W[e].astype(_F16).reshape(NT, NSZ, KD, P).transpose(3, 0, 2, 1)
        )
        bias_np = np.ascontiguousarray(np.broadcast_to(b[e], (P, D)).astype(np.float32))
        s_tok = np.zeros(C, dtype=np.float32)
        for k in range(TOP_K):
            sel = idx[tok, k] == e
            s_tok[:cnt][sel] = vals[tok[sel], k]
        scale_np = np.ascontiguousarray(s_tok.reshape(m_tiles, P).T)
        in_maps.append({"xt": xt_np, "wt": wt_np, "bias": bias_np, "scale": scale_np})
    return in_maps, token_lists, counts, m_tiles


def kernel(x, W, b, Wg, bg):
    from concourse.bass_utils import run_bass_kernel_spmd

    x = np.asarray(x, dtype=np.float32)
    W = np.asarray(W, dtype=np.float32)
    b = np.asarray(b, dtype=np.float32)
    Wg = np.asarray(Wg, dtype=np.float32)
    bg = np.asarray(bg, dtype=np.float32)

    idx, vals = _routing(x, Wg, bg)
    in_maps, token_lists, counts, m_tiles = _prep_inputs(x, W, b, idx, vals)
    nc = _get_nc(m_tiles)
    res = run_bass_kernel_spmd(nc, in_maps, core_ids=list(range(N_CORES)))

    out = np.zeros((B, D), dtype=np.float32)
    for e in range(E):
        ye = res.results[e]["y"]
        out[token_lists[e]] += ye[:counts[e]]
    return out


# revision 6
# speedup vs baseline: 1.1160x; 1.1160x over previous
"""MoE (top-2 of 8 experts, B=8192, D=2048) on 8 Trainium2 NeuronCores.

Strategy (expert-parallel, per sharding hint): the host computes the gate
softmax + top-2 routing (float64 numpy; rank-2/3 margins are ~3e-5 so the
selection matches any f32 reference platform), dispatches each token's rows
to its experts' cores, and each core computes
    y_e = relu(x_e @ W[e].T + b[e]) * gate_scale
for its gathered tokens as an fp16 tiled matmul on the PE array.  The host
then scatter-adds the (at most 2) expert contributions per token.

Device work per core: ~C*D*D*2 ~ 18 GFLOP fp16 (vs 8x that for the dense
reference), PSUM-accumulated over the 2048-deep contraction in 16 chunks of
128, with bias-add (VectorE) + relu*scale (ScalarE, per-partition scale)
epilogue fused off the critical PE path.  Schedule: a PE warmup burst and an
n=0 sweep start compute after only ~2.5MB of DMA; the other 6MB of weights
and 8.4MB of tokens stream in underneath (xt on the ACT DGE ring, wt on the
SP ring).  Measured ~0.3ms/core steady-state vs a ~0.23ms PE streaming bound
(the gap is mostly per-matmul self-LDWEIGHTS, which this walrus cannot
elide).
"""

import math

import numpy as np

B, D, E, TOP_K = 8192, 2048, 8, 2
N_CORES = 8
P = 128
KD = D // P  # 16 contraction chunks
NT = 4
NSZ = D // NT  # 512 output columns per psum tile

_F16 = np.float16

_nc_cache = {}


def _routing(x, Wg, bg):
    """Gate softmax + top-2 in float64; returns (idx [B,2] int, vals [B,2] f32)."""
    logits = x.astype(np.float64) @ Wg.astype(np.float64).T + bg.astype(np.float64)
    logits -= logits.max(-1, keepdims=True)
    eL = np.exp(logits)
    gate = eL / eL.sum(-1, keepdims=True)
    order = np.argsort(-gate, axis=-1, kind="stable")
    idx = order[:, :TOP_K]
    vals = np.take_along_axis(gate, idx, -1).astype(np.float32)
    return idx, vals


def _build(m_tiles, reps=1):
    """Build + compile the per-core Bass kernel for C = m_tiles*128 tokens."""
    import concourse.mybir as mybir
    import concourse.tile as tile
    from concourse import bacc

    nc = bacc.Bacc("TRN2", target_bir_lowering=False)
    C = m_tiles * P
    xt = nc.dram_tensor("xt", [P, m_tiles, KD, P], mybir.dt.float16, kind="ExternalInput")
    wt = nc.dram_tensor("wt", [P, NT, KD, NSZ], mybir.dt.float16, kind="ExternalInput")
    bias = nc.dram_tensor("bias", [P, D], mybir.dt.float32, kind="ExternalInput")
    scale = nc.dram_tensor("scale", [P, m_tiles], mybir.dt.float32, kind="ExternalInput")
    y = nc.dram_tensor("y", [C, D], mybir.dt.float32, kind="ExternalOutput")

    with tile.TileContext(nc) as tc:
        with (
            tc.tile_pool(name="wp", bufs=1) as wp,
            tc.tile_pool(name="xp", bufs=1) as xp,
            tc.tile_pool(name="cp", bufs=1) as cp,
            tc.tile_pool(name="op", bufs=6) as op_,
            tc.tile_pool(name="pp", bufs=8, space="PSUM") as pp,
        ):
            def epilogue(ps, bias_sb, scale_sb, m, n):
                ot = op_.tile([P, NSZ], mybir.dt.float32, tag="ot", name="ot")
                nc.vector.tensor_tensor(
                    ot[:], ps[:], bias_sb[:, n * NSZ:(n + 1) * NSZ], mybir.AluOpType.add
                )
                nc.scalar.activation(
                    ot[:], ot[:], mybir.ActivationFunctionType.Relu,
                    scale=scale_sb[:, m:m + 1],
                )
                nc.sync.dma_start(y[m * P:(m + 1) * P, n * NSZ:(n + 1) * NSZ], ot[:])

            def body(_i=None):
                bias_sb = cp.tile([P, D], mybir.dt.float32, tag="bias", name="bias_sb")
                nc.sync.dma_start(bias_sb[:], bias[:])
                scale_sb = cp.tile([P, m_tiles], mybir.dt.float32, tag="scale", name="scale_sb")
                nc.sync.dma_start(scale_sb[:], scale[:])

                wts = [None] * NT
                wts[0] = wp.tile([P, KD, NSZ], mybir.dt.float16, tag="wt0", name="wt_sb0")
                nc.sync.dma_start(wts[0][:], wt[:, 0])

                # PE warmup while the first weight DMA is in flight: ~4us of
                # garbage matmuls un-throttle the HAM clock gate (1.2->2.4GHz)
                # so the real sweep starts warm.
                warm = cp.tile([P, 640], mybir.dt.float16, tag="warm", name="warm")
                nc.vector.memset(warm[:], 0.0)
                wps = pp.tile([P, NSZ], mybir.dt.float32, tag="ps", name="warmps")
                for _w in range(16):
                    nc.tensor.matmul(wps[:], warm[:, 0:P], warm[:, P:P + NSZ],
                                     start=True, stop=True)

                # Phase 1: n=0 sweep over all m-tiles.  PE starts after only
                # wt[0] (2MB) + xt[0] (0.5MB) land; the remaining weight and
                # token DMAs hide under this ~60us sweep.  xt loads go via the
                # ACT HWDGE ring so they don't FIFO behind wt on the SP ring.
                xts = [None] * m_tiles
                for m in range(m_tiles):
                    xts[m] = xp.tile([P, KD, P], mybir.dt.float16, tag=f"xt{m}", name=f"xt_sb{m}")
                    nc.scalar.dma_start(xts[m][:], xt[:, m])
                    ps = pp.tile([P, NSZ], mybir.dt.float32, tag="ps", name="ps")
                    for kd in range(KD):
                        nc.tensor.matmul(
                            ps[:], xts[m][:, kd], wts[0][:, kd],
                            start=(kd == 0), stop=(kd == KD - 1),
                        )
                    epilogue(ps, bias_sb, scale_sb, m, 0)
                for n in range(1, NT):
                    t = wp.tile([P, KD, NSZ], mybir.dt.float16, tag=f"wt{n}", name=f"wt_sb{n}")
                    nc.sync.dma_start(t[:], wt[:, n])
                    wts[n] = t
                # Phase 2: n=1..3 with 3 psum banks per m-tile.
                for m in range(m_tiles):
                    pss = [pp.tile([P, NSZ], mybir.dt.float32, tag="ps", name="ps") for _ in range(NT - 1)]
                    for kd in range(KD):
                        for j, n in enumerate(range(1, NT)):
                            nc.tensor.matmul(
                                pss[j][:], xts[m][:, kd], wts[n][:, kd],
                                start=(kd == 0), stop=(kd == KD - 1),
                            )
                    for j, n in enumerate(range(1, NT)):
                        epilogue(pss[j], bias_sb, scale_sb, m, n)

            if reps == 1:
                body()
            else:
                with tc.For_i(0, reps) as _i:
                    body(_i)

    nc.compile()
    return nc


def _get_nc(m_tiles, reps=1):
    key = (m_tiles, reps)
    if key not in _nc_cache:
        _nc_cache[key] = _build(m_tiles, reps)
    return _nc_cache[key]


def _prep_inputs(x, W, b, idx, vals):
    """Per-core input maps: blocked bf16 xT/wT layouts + bias/scale tiles."""
    in_maps = []
    token_lists = []
    counts = []
    for e in range(E):
        tok = np.where((idx == e).any(axis=1))[0]
        token_lists.append(tok)
        counts.append(len(tok))
    c_max = max(counts)
    m_tiles = max(1, math.ceil(c_max / P))
    C = m_tiles * P

    for e in range(E):
        tok = token_lists[e]
        cnt = len(tok)
        Xp = np.zeros((C, D), dtype=_F16)
        Xp[:cnt] = x[tok].astype(_F16)
        xt_np = np.ascontiguousarray(
            Xp.reshape(m_tiles, P, KD, P).transpose(3, 0, 2, 1)
        )
        wt_np = np.ascontiguousarray(
            W[e].astype(_F16).reshape(NT, NSZ, KD, P).transpose(3, 0, 2, 1)
        )
        bias_np = np.ascontiguousarray(np.broadcast_to(b[e], (P, D)).astype(np.float32))
        s_tok = np.zeros(C, dtype=np.float32)
        for k in range(TOP_K):
            sel = idx[tok, k] == e
            s_tok[:cnt][sel] = vals[tok[sel], k]
        scale_np = np.ascontiguousarray(s_tok.reshape(m_tiles, P).T)
        in_maps.append({"xt": xt_np, "wt": wt_np, "bias": bias_np, "scale": scale_np})
    return in_maps, token_lists, counts, m_tiles


def kernel(x, W, b, Wg, bg):
    from concourse.bass_utils import run_bass_kernel_spmd

    x = np.asarray(x, dtype=np.float32)
    W = np.asarray(W, dtype=np.float32)
    b = np.asarray(b, dtype=np.float32)
    Wg = np.asarray(Wg, dtype=np.float32)
    bg = np.asarray(bg, dtype=np.float32)

    idx, vals = _routing(x, Wg, bg)
    in_maps, token_lists, counts, m_tiles = _prep_inputs(x, W, b, idx, vals)
    nc = _get_nc(m_tiles)
    res = run_bass_kernel_spmd(nc, in_maps, core_ids=list(range(N_CORES)))

    out = np.zeros((B, D), dtype=np.float32)
    for e in range(E):
        ye = res.results[e]["y"]
        out[token_lists[e]] += ye[:counts[e]]
    return out
